# revision 4
# baseline (speedup 1.0000x reference)
"""DenseSNN Trainium2 kernel: 4-layer LIF SNN, T=100 steps, B=128, D=H=2048, C=100.

Strategy
--------
Feed-forward unroll into per-layer phases (layer-l spikes at step t depend only
on layer-(l-1) spikes at steps <= t):

    CUR1 = x @ W1 + b1          (batched over all T*B rows)
    S1   = LIF-scan_T(CUR1)     (elementwise in (B,H), sequential in T)
    ... repeat for W2, W3, Wo; output = sum_t spikes of the last layer.

Pure data-parallel over batch: 16 samples/core on 8 cores, no collectives.

On-chip layout: activations transposed [feature -> 16 chunks x 128 partitions,
(t,b) -> free axis]. Matmuls are fp8e4 DoubleRow (2 k-tiles per instruction).
Weights are host-prescaled by 512; the PSUM->SBUF drain (scalar engine)
descales and adds the bias.

v2 performance structure (from HW trace analysis of v1 @603us):
- The LIF scan (3 DVE ops/step/layer) is the near-critical chain. Its solo
  cadence is ~880ns/step when nothing else contends; the v1 per-step gpsimd
  CAST export (c-major S, 16B-chunk scatter) dragged it to ~1510ns/step.
- Fix: spike tiles S are TIME-MAJOR [t][c][b], identical to the scan's ring
  layout, so the export becomes one contiguous [128,1024] bf16->fp8 copy per
  4 steps on gpsimd. Matmuls read S through a 4D rhs access pattern
  [k, 2, t, b] (verified bit-exact in DoubleRow mode on HW).
- Row blocks per layer: 512 + 576 + 512 (= T*BC rows). The 576 block issues
  N=512 + N=64 matmul pairs sharing one LDWEIGHTS-load (275ns/group measured),
  so no exposed weight-load stalls anywhere.
- Output layer: blocks 0-1 of its scan run on gpsimd (concurrent with the
  layer-3 DVE scan); the last block runs on DVE to shorten the tail. The
  spike-count accumulation runs on gpsimd off the critical chain.
"""

import numpy as np
import ml_dtypes

import concourse.bass as bass
import concourse.mybir as mybir
import concourse.tile as tile
from concourse import bacc
from concourse.bass_utils import run_bass_kernel_spmd

# Problem constants (hardcoded per contract)
T, B, D, H, C = 100, 128, 2048, 2048, 100
NCORES = 8
BC = B // NCORES          # 16 samples per core
R = T * BC                # 1600 rows (t,b) per core
KC = D // 128             # 16 contraction chunks
KP = KC // 2              # 8 DoubleRow chunk-pairs
HC = H // 128             # 16 output-feature chunks
BETA = 0.9
WSCALE = 512.0            # host-side fp8 weight scale; descaled at drain
RD = 8                    # spike ring depth (2 export blocks of 4 steps)

# Step blocks per layer: (step0, nsteps). Middle block carries the 64-row
# tail as an LDW-sharing matmul pair (N=512 + N=64).
BLOCKS = [(0, 32), (32, 36), (68, 32)]

F32 = mybir.dt.float32
BF16 = mybir.dt.bfloat16
F8 = mybir.dt.float8e4
ALU = mybir.AluOpType
ACTF = mybir.ActivationFunctionType
DROW = mybir.MatmulPerfMode.DoubleRow


def _build_nc():
    nc = bacc.Bacc("TRN2", target_bir_lowering=False)

    xT_d = nc.dram_tensor("xT", [KC, 128, R], F8, kind="ExternalInput")
    w_d = [
        nc.dram_tensor("w1", [D, H], F8, kind="ExternalInput"),
        nc.dram_tensor("w2", [H, H], F8, kind="ExternalInput"),
        nc.dram_tensor("w3", [H, H], F8, kind="ExternalInput"),
    ]
    wo_d = nc.dram_tensor("wo", [H, 128], F8, kind="ExternalInput")  # C pad 128
    bias_d = nc.dram_tensor("biases", [128, 3 * HC], F32, kind="ExternalInput")
    bo_d = nc.dram_tensor("biaso", [C, 1], F32, kind="ExternalInput")
    out_d = nc.dram_tensor("out", [C, BC], F32, kind="ExternalOutput")

    with tile.TileContext(nc) as tc:
        with (
            tc.tile_pool(name="spool", bufs=2) as spool,
            tc.tile_pool(name="wpool", bufs=2) as wpool,
            tc.tile_pool(name="xpool", bufs=1) as xpool,
            tc.tile_pool(name="cpool", bufs=2) as cpool,
            tc.tile_pool(name="opool", bufs=2) as opool,
            tc.tile_pool(name="small", bufs=1) as small,
            tc.tile_pool(name="psb", bufs=4, space="PSUM") as psb,
            tc.tile_pool(name="pst", bufs=2, space="PSUM") as pst,
        ):
            # Spike tensors, TIME-MAJOR: [128part(k-in), (t, c, b)] fp8.
            # S3 reuses S1's slot (S1 is dead once layer-2 matmuls finish).
            S1 = spool.tile([128, T * KC * BC], F8, tag="S")  # 25600/partition
            S2 = spool.tile([128, T * KC * BC], F8, tag="S")
            S3 = spool.tile([128, T * KC * BC], F8, tag="S")
            w_sb = [
                wpool.tile([128, KC * H], F8, tag="W", name=f"w{i}_sb")
                for i in range(3)
            ]                                  # w3 reuses w1's slot
            wo_sb = small.tile([128, KC * 128], F8)

            # LIF state (bf16, 2x DVE mode). Ring is 8 deep; export copies a
            # contiguous 4-slot block to S every 4 steps.
            stm = small.tile([128, 9 * 256], BF16)   # 3 layers x (2 pp + tmp)
            mem_pp = [
                [stm[:, (3 * li + pp) * 256:(3 * li + pp + 1) * 256]
                 for pp in range(2)]
                for li in range(3)
            ]
            mem_t = [
                stm[:, (3 * li + 2) * 256:(3 * li + 3) * 256] for li in range(3)
            ]
            sring = small.tile([128, 3 * RD * 256], BF16)

            # fp32 small state: biases + output layer
            st = small.tile([128, 224], F32)
            bias_sb = st[:, 0:48]               # [128,48] = 3 layers x 16 chunks
            memo = st[:100, 48:64]              # [100, 16]
            ssum = st[:100, 64:80]
            bo_sb = st[:100, 80:81]             # [100, 1]
            so_ring = st[:100, 96:224]          # [100, 8*16] spike ring

            nc.gpsimd.memset(st[:], 0.0)
            nc.gpsimd.memset(stm[:], 0.0)
            nc.gpsimd.memset(sring[:], 0.0)
            nc.sync.dma_start(bias_sb, bias_d[:])
            nc.sync.dma_start(bo_sb, bo_d[:])
            for kc in range(KC):
                nc.sync.dma_start(
                    wo_sb[:, kc * 128:(kc + 1) * 128],
                    wo_d[kc * 128:(kc + 1) * 128, :],
                )

            def load_w(li):
                w = w_sb[li]
                for kc in range(KC):
                    nc.sync.dma_start(
                        w[:, kc * H:(kc + 1) * H],
                        w_d[li][kc * 128:(kc + 1) * 128, :],
                    )

            def mm_group(w3d, rhs_big, rhs_tail, drain_big, drain_tail, nhc,
                         nst):
                """Matmuls + drains for one (layer, block). nhc output chunks;
                block has nst steps: 32 big-only, or 36 = 32 big + 4 tail."""
                nr = 512
                for hc in range(nhc):
                    lo, hi = hc * 128, hc * 128 + 128
                    ps = psb.tile([128, nr], F32, tag="ps", name="ps")
                    pt = None
                    if nst == 36:
                        pt = pst.tile([128, 64], F32, tag="pt", name="pt")
                    for kp in range(KP):
                        nc.tensor.matmul(
                            ps[:], w3d[:, 2 * kp:2 * kp + 2, lo:hi],
                            rhs_big(kp),
                            start=(kp == 0), stop=(kp == KP - 1),
                            perf_mode=DROW,
                        )
                        if pt is not None:
                            nc.tensor.matmul(
                                pt[:], w3d[:, 2 * kp:2 * kp + 2, lo:hi],
                                rhs_tail(kp),
                                start=(kp == 0), stop=(kp == KP - 1),
                                perf_mode=DROW,
                            )
                    drain_big(hc, ps)
                    if pt is not None:
                        drain_tail(hc, pt)

            def hidden_layer(li, rhs_of, S_out):
                """One hidden layer: per block, matmuls + LIF scan steps."""
                mpp = mem_pp[li]
                mt = mem_t[li]
                ring = sring[:, li * RD * 256:(li + 1) * RD * 256]
                w3d = w_sb[li].rearrange("p (c h) -> p c h", c=KC)
                for bi, (t0, nst) in enumerate(BLOCKS):
                    rhs_big, rhs_tail = rhs_of(bi)
                    cur = cpool.tile([128, 36 * 256], BF16, tag="cur",
                                     name="cur")
                    curT = cur.rearrange("p (t cb) -> p t cb", cb=256)

                    def drain_big(hc, ps, curT=curT):
                        nc.scalar.activation(
                            curT[:, :32, hc * BC:(hc + 1) * BC],
                            ps[:].rearrange("p (t b) -> p t b", b=BC),
                            ACTF.Identity,
                            bias=bias_sb[:, li * HC + hc:li * HC + hc + 1],
                            scale=1.0 / WSCALE,
                        )

                    def drain_tail(hc, pt, curT=curT):
                        nc.scalar.activation(
                            curT[:, 32:36, hc * BC:(hc + 1) * BC],
                            pt[:].rearrange("p (t b) -> p t b", b=BC),
                            ACTF.Identity,
                            bias=bias_sb[:, li * HC + hc:li * HC + hc + 1],
                            scale=1.0 / WSCALE,
                        )

                    mm_group(w3d, rhs_big, rhs_tail, drain_big, drain_tail,
                             HC, nst)

                    for tl in range(nst):
                        t = t0 + tl
                        cur_t = cur[:, tl * 256:(tl + 1) * 256]
                        sp_c = ring[:, ((t - 1) % RD) * 256:
                                    ((t - 1) % RD + 1) * 256]
                        sn_c = ring[:, (t % RD) * 256:(t % RD + 1) * 256]
                        m_prev = mpp[(t - 1) % 2]
                        m_cur = mpp[t % 2]
                        # tmp = beta*mem + cur
                        nc.vector.scalar_tensor_tensor(
                            mt, m_prev, BETA, cur_t, ALU.mult, ALU.add
                        )
                        # mem_new = tmp - s_prev (reset-by-subtraction;
                        # ring slot 7 holds zeros at t=0)
                        nc.vector.tensor_tensor(m_cur, mt, sp_c, ALU.subtract)
                        # spike = mem_new > 1
                        nc.vector.tensor_scalar(
                            sn_c, m_cur, 1.0, None, ALU.is_gt
                        )
                        if t % 4 == 3:
                            # contiguous 4-step block export to time-major S
                            blk = ((t // 4) % 2) * 1024
                            nc.gpsimd.tensor_copy(
                                out=S_out[:, (t - 3) * 256:(t + 1) * 256],
                                in_=ring[:, blk:blk + 1024],
                            )

            # ---- rhs providers
            def rhs_of_l1(bi):
                t0, nst = BLOCKS[bi]
                r0 = t0 * BC
                nr_all = nst * BC
                xin = xpool.tile([128, KC * 576], F8, tag="xin", name="xin")
                for kc in range(KC):
                    nc.sync.dma_start(
                        xin[:, kc * nr_all:(kc + 1) * nr_all],
                        xT_d[kc][:, r0:r0 + nr_all],
                    )
                x3 = xin[:, :KC * nr_all].rearrange("p (c r) -> p c r", c=KC)
                rhs_big = lambda kp: x3[:, 2 * kp:2 * kp + 2, 0:512]
                rhs_tail = lambda kp: x3[:, 2 * kp:2 * kp + 2, 512:576]
                return rhs_big, rhs_tail

            def rhs_of_S(S_in):
                S4 = S_in.rearrange("p (t c b) -> p c t b", t=T, c=KC)

                def f(bi):
                    t0, nst = BLOCKS[bi]
                    rhs_big = lambda kp: S4[:, 2 * kp:2 * kp + 2,
                                            t0:t0 + 32, :]
                    rhs_tail = lambda kp: S4[:, 2 * kp:2 * kp + 2,
                                             t0 + 32:t0 + 36, :]
                    return rhs_big, rhs_tail
                return f

            # ---- network
            load_w(0)
            load_w(1)
            hidden_layer(0, rhs_of_l1, S1)
            load_w(2)
            hidden_layer(1, rhs_of_S(S1), S2)
            hidden_layer(2, rhs_of_S(S2), S3)

            # ---- Output layer. Scan blocks 0-1 on gpsimd (concurrent with
            # the layer-3 DVE scan), block 2 on DVE (short tail). Spike-count
            # accumulation on gpsimd.
            wo3d = wo_sb.rearrange("p (c h) -> p c h", c=KC)
            S3_4 = S3.rearrange("p (t c b) -> p c t b", t=T, c=KC)
            for bi, (t0, nst) in enumerate(BLOCKS):
                rhs_big = lambda kp, t0=t0: S3_4[:, 2 * kp:2 * kp + 2,
                                                 t0:t0 + 32, :]
                rhs_tail = lambda kp, t0=t0: S3_4[:, 2 * kp:2 * kp + 2,
                                                  t0 + 32:t0 + 36, :]
                curo = opool.tile([128, 576], BF16, tag="curo", name="curo")

                def drain_big(hc, ps, curo=curo):
                    nc.scalar.activation(
                        curo[:100, 0:512], ps[:100, :], ACTF.Identity,
                        bias=bo_sb, scale=1.0 / WSCALE,
                    )

                def drain_tail(hc, pt, curo=curo):
                    nc.scalar.activation(
                        curo[:100, 512:576], pt[:100, :], ACTF.Identity,
                        bias=bo_sb, scale=1.0 / WSCALE,
                    )

                mm_group(wo3d, rhs_big, rhs_tail, drain_big, drain_tail,
                         1, nst)

                eng = nc.vector
                for tl in range(nst):
                    t = t0 + tl
                    cur_t = curo[:100, tl * BC:(tl + 1) * BC]
                    so_prev = so_ring[:, ((t - 1) % 8) * BC:
                                      ((t - 1) % 8 + 1) * BC]
                    so_new = so_ring[:, (t % 8) * BC:(t % 8 + 1) * BC]
                    eng.scalar_tensor_tensor(
                        memo, memo, BETA, cur_t, ALU.mult, ALU.add
                    )
                    eng.scalar_tensor_tensor(
                        so_new, memo, 1.0, so_prev, ALU.subtract, ALU.is_gt
                    )
                    eng.tensor_tensor(memo, memo, so_prev, ALU.subtract)
                    nc.gpsimd.tensor_tensor(ssum, ssum, so_new, ALU.add)

            nc.sync.dma_start(out_d[:], ssum)

    nc.compile()
    return nc


_NC_CACHE = None


def _get_nc():
    global _NC_CACHE
    if _NC_CACHE is None:
        _NC_CACHE = _build_nc()
    return _NC_CACHE


def make_in_maps(x_seq, W1, b1, W2, b2, W3, b3, Wo, bo):
    f8 = ml_dtypes.float8_e4m3
    w1 = np.ascontiguousarray((W1 * WSCALE).astype(f8))
    w2 = np.ascontiguousarray((W2 * WSCALE).astype(f8))
    w3 = np.ascontiguousarray((W3 * WSCALE).astype(f8))
    wo_pad = np.zeros((H, 128), np.float32)
    wo_pad[:, :C] = Wo * WSCALE
    wo = np.ascontiguousarray(wo_pad.astype(f8))
    biases = np.concatenate(
        [b.reshape(HC, 128).T for b in (b1, b2, b3)], axis=1
    ).astype(np.float32)                       # [128, 48]
    biases = np.ascontiguousarray(biases)
    bo_a = np.ascontiguousarray(bo.reshape(C, 1).astype(np.float32))
    in_maps = []
    for c in range(NCORES):
        xs = x_seq[:, c * BC:(c + 1) * BC, :]              # [T, BC, D]
        xT = xs.transpose(2, 0, 1).reshape(KC, 128, R)     # [D,(t,b)] chunked
        in_maps.append({
            "xT": np.ascontiguousarray(xT.astype(f8)),
            "w1": w1, "w2": w2, "w3": w3, "wo": wo,
            "biases": biases, "biaso": bo_a,
        })
    return in_maps


def kernel(x_seq, W1, b1, W2, b2, W3, b3, Wo, bo):
    nc = _get_nc()
    in_maps = make_in_maps(x_seq, W1, b1, W2, b2, W3, b3, Wo, bo)
    res = run_bass_kernel_spmd(nc, in_maps, core_ids=list(range(NCORES)))
    outs = [res.results[c]["out"] for c in range(NCORES)]   # each [C, BC]
    return np.concatenate([o.T for o in outs], axis=0).astype(np.float32)


# revision 5
# speedup vs baseline: 1.3315x; 1.3315x over previous
"""DenseSNN Trainium2 kernel: 4-layer LIF SNN, T=100 steps, B=128, D=H=2048, C=100.

Strategy
--------
Feed-forward unroll into per-layer phases (layer-l spikes at step t depend only
on layer-(l-1) spikes at steps <= t):

    CUR1 = x @ W1 + b1          (batched over all T*B rows)
    S1   = LIF-scan_T(CUR1)     (elementwise in (B,H), sequential in T)
    ... repeat for W2, W3, Wo; output = sum_t spikes of the last layer.

Pure data-parallel over batch: 16 samples/core on 8 cores, no collectives.

On-chip layout: activations transposed [feature -> 16 chunks x 128 partitions,
(t,b) -> free axis]. Matmuls are fp8e4 DoubleRow (2 k-tiles per instruction).
Weights are host-prescaled by 512; the PSUM->SBUF drain (scalar engine)
descales and adds the bias.

v2 performance structure (from HW trace analysis of v1 @603us):
- The LIF scan (3 DVE ops/step/layer) is the near-critical chain. Its solo
  cadence is ~880ns/step when nothing else contends; the v1 per-step gpsimd
  CAST export (c-major S, 16B-chunk scatter) dragged it to ~1510ns/step.
- Fix: spike tiles S are TIME-MAJOR [t][c][b], identical to the scan's ring
  layout, so the export becomes one contiguous [128,1024] bf16->fp8 copy per
  4 steps on gpsimd. Matmuls read S through a 4D rhs access pattern
  [k, 2, t, b] (verified bit-exact in DoubleRow mode on HW).
- Row blocks per layer: 512 + 576 + 512 (= T*BC rows). The 576 block issues
  N=512 + N=64 matmul pairs sharing one LDWEIGHTS-load (275ns/group measured),
  so no exposed weight-load stalls anywhere.
- Output layer: blocks 0-1 of its scan run on gpsimd (concurrent with the
  layer-3 DVE scan); the last block runs on DVE to shorten the tail. The
  spike-count accumulation runs on gpsimd off the critical chain.
"""

import numpy as np
import ml_dtypes

import concourse.bass as bass
import concourse.mybir as mybir
import concourse.tile as tile
from concourse import bacc
from concourse.bass_utils import run_bass_kernel_spmd

# Problem constants (hardcoded per contract)
T, B, D, H, C = 100, 128, 2048, 2048, 100
NCORES = 8
BC = B // NCORES          # 16 samples per core
R = T * BC                # 1600 rows (t,b) per core
KC = D // 128             # 16 contraction chunks
KP = KC // 2              # 8 DoubleRow chunk-pairs
HC = H // 128             # 16 output-feature chunks
BETA = 0.9
WSCALE = 512.0            # host-side fp8 weight scale; descaled at drain
RD = 8                    # spike ring depth (2 export blocks of 4 steps)

# Step blocks per layer: (step0, nsteps). Middle block carries the 64-row
# tail as an LDW-sharing matmul pair (N=512 + N=64).
BLOCKS = [(0, 32), (32, 36), (68, 32)]

F32 = mybir.dt.float32
BF16 = mybir.dt.bfloat16
F8 = mybir.dt.float8e4
ALU = mybir.AluOpType
ACTF = mybir.ActivationFunctionType
DROW = mybir.MatmulPerfMode.DoubleRow


def _build_nc():
    nc = bacc.Bacc("TRN2", target_bir_lowering=False)

    xT_d = nc.dram_tensor("xT", [KC, 128, R], F8, kind="ExternalInput")
    w_d = [
        nc.dram_tensor("w1", [D, H], F8, kind="ExternalInput"),
        nc.dram_tensor("w2", [H, H], F8, kind="ExternalInput"),
        nc.dram_tensor("w3", [H, H], F8, kind="ExternalInput"),
    ]
    wo_d = nc.dram_tensor("wo", [H, 128], F8, kind="ExternalInput")  # C pad 128
    bias_d = nc.dram_tensor("biases", [128, 3 * HC], F32, kind="ExternalInput")
    bo_d = nc.dram_tensor("biaso", [C, 1], F32, kind="ExternalInput")
    out_d = nc.dram_tensor("out", [C, BC], F32, kind="ExternalOutput")

    with tile.TileContext(nc) as tc:
        with (
            tc.tile_pool(name="spool", bufs=2) as spool,
            tc.tile_pool(name="wpool", bufs=2) as wpool,
            tc.tile_pool(name="xpool", bufs=1) as xpool,
            tc.tile_pool(name="cpool", bufs=2) as cpool,
            tc.tile_pool(name="opool", bufs=2) as opool,
            tc.tile_pool(name="small", bufs=1) as small,
            tc.tile_pool(name="psb", bufs=4, space="PSUM") as psb,
            tc.tile_pool(name="pst", bufs=2, space="PSUM") as pst,
        ):
            # Spike tensors, TIME-MAJOR: [128part(k-in), (t, c, b)] fp8.
            # S3 reuses S1's slot (S1 is dead once layer-2 matmuls finish).
            S1 = spool.tile([128, T * KC * BC], F8, tag="S")  # 25600/partition
            S2 = spool.tile([128, T * KC * BC], F8, tag="S")
            S3 = spool.tile([128, T * KC * BC], F8, tag="S")
            w_sb = [
                wpool.tile([128, KC * H], F8, tag="W", name=f"w{i}_sb")
                for i in range(3)
            ]                                  # w3 reuses w1's slot
            wo_sb = small.tile([128, KC * 128], F8)

            # LIF state (bf16, 2x DVE mode). Ring is 8 deep; export copies a
            # contiguous 4-slot block to S every 4 steps.
            stm = small.tile([128, 9 * 256], BF16)   # 3 layers x (2 pp + tmp)
            mem_pp = [
                [stm[:, (3 * li + pp) * 256:(3 * li + pp + 1) * 256]
                 for pp in range(2)]
                for li in range(3)
            ]
            mem_t = [
                stm[:, (3 * li + 2) * 256:(3 * li + 3) * 256] for li in range(3)
            ]
            sring = small.tile([128, 3 * RD * 256], BF16)

            # fp32 small state: biases + output layer
            st = small.tile([128, 224], F32)
            bias_sb = st[:, 0:48]               # [128,48] = 3 layers x 16 chunks
            memo = st[:100, 48:64]              # [100, 16]
            ssum = st[:100, 64:80]
            bo_sb = st[:100, 80:81]             # [100, 1]
            so_ring = st[:100, 96:224]          # [100, 8*16] spike ring

            nc.gpsimd.memset(st[:], 0.0)
            nc.gpsimd.memset(stm[:], 0.0)
            nc.gpsimd.memset(sring[:], 0.0)
            nc.sync.dma_start(bias_sb, bias_d[:])
            nc.sync.dma_start(bo_sb, bo_d[:])
            for kc in range(KC):
                nc.sync.dma_start(
                    wo_sb[:, kc * 128:(kc + 1) * 128],
                    wo_d[kc * 128:(kc + 1) * 128, :],
                )

            def load_w(li):
                w = w_sb[li]
                for kc in range(KC):
                    nc.sync.dma_start(
                        w[:, kc * H:(kc + 1) * H],
                        w_d[li][kc * 128:(kc + 1) * 128, :],
                    )

            def mm_group(w3d, rhs_big, rhs_tail, drain_big, drain_tail, nhc,
                         nst):
                """Matmuls + drains for one (layer, block). nhc output chunks;
                block has nst steps: 32 big-only, or 36 = 32 big + 4 tail."""
                nr = 512
                for hc in range(nhc):
                    lo, hi = hc * 128, hc * 128 + 128
                    ps = psb.tile([128, nr], F32, tag="ps", name="ps")
                    pt = None
                    if nst == 36:
                        pt = pst.tile([128, 64], F32, tag="pt", name="pt")
                    for kp in range(KP):
                        nc.tensor.matmul(
                            ps[:], w3d[:, 2 * kp:2 * kp + 2, lo:hi],
                            rhs_big(kp),
                            start=(kp == 0), stop=(kp == KP - 1),
                            perf_mode=DROW,
                        )
                        if pt is not None:
                            nc.tensor.matmul(
                                pt[:], w3d[:, 2 * kp:2 * kp + 2, lo:hi],
                                rhs_tail(kp),
                                start=(kp == 0), stop=(kp == KP - 1),
                                perf_mode=DROW,
                            )
                    drain_big(hc, ps)
                    if pt is not None:
                        drain_tail(hc, pt)

            def hidden_layer(li, rhs_of, S_out):
                """One hidden layer: per block, matmuls + LIF scan steps."""
                mpp = mem_pp[li]
                mt = mem_t[li]
                ring = sring[:, li * RD * 256:(li + 1) * RD * 256]
                w3d = w_sb[li].rearrange("p (c h) -> p c h", c=KC)
                for bi, (t0, nst) in enumerate(BLOCKS):
                    rhs_big, rhs_tail = rhs_of(bi)
                    cur = cpool.tile([128, 36 * 256], BF16, tag="cur",
                                     name="cur")
                    curT = cur.rearrange("p (t cb) -> p t cb", cb=256)

                    def drain_big(hc, ps, curT=curT):
                        nc.scalar.activation(
                            curT[:, :32, hc * BC:(hc + 1) * BC],
                            ps[:].rearrange("p (t b) -> p t b", b=BC),
                            ACTF.Identity,
                            bias=bias_sb[:, li * HC + hc:li * HC + hc + 1],
                            scale=1.0 / WSCALE,
                        )

                    def drain_tail(hc, pt, curT=curT):
                        nc.scalar.activation(
                            curT[:, 32:36, hc * BC:(hc + 1) * BC],
                            pt[:].rearrange("p (t b) -> p t b", b=BC),
                            ACTF.Identity,
                            bias=bias_sb[:, li * HC + hc:li * HC + hc + 1],
                            scale=1.0 / WSCALE,
                        )

                    mm_group(w3d, rhs_big, rhs_tail, drain_big, drain_tail,
                             HC, nst)

                    for tl in range(nst):
                        t = t0 + tl
                        cur_t = cur[:, tl * 256:(tl + 1) * 256]
                        sp_c = ring[:, ((t - 1) % RD) * 256:
                                    ((t - 1) % RD + 1) * 256]
                        sn_c = ring[:, (t % RD) * 256:(t % RD + 1) * 256]
                        m_prev = mpp[(t - 1) % 2]
                        m_cur = mpp[t % 2]
                        # tmp = beta*mem + cur
                        nc.vector.scalar_tensor_tensor(
                            mt, m_prev, BETA, cur_t, ALU.mult, ALU.add
                        )
                        # mem_new = tmp - s_prev (reset-by-subtraction;
                        # ring slot 7 holds zeros at t=0)
                        nc.vector.tensor_tensor(m_cur, mt, sp_c, ALU.subtract)
                        # spike = mem_new > 1
                        nc.vector.tensor_scalar(
                            sn_c, m_cur, 1.0, None, ALU.is_gt
                        )
                        if t % 4 == 3:
                            # contiguous 4-step block export to time-major S:
                            # gpsimd-initiated DMA with bf16->fp8 cast (data
                            # movement rides the idle DMA engines)
                            blk = ((t // 4) % 2) * 1024
                            nc.gpsimd.dma_start(
                                S_out[:, (t - 3) * 256:(t + 1) * 256],
                                ring[:, blk:blk + 1024],
                            )

            # ---- rhs providers
            def rhs_of_l1(bi):
                t0, nst = BLOCKS[bi]
                r0 = t0 * BC
                nr_all = nst * BC
                xin = xpool.tile([128, KC * 576], F8, tag="xin", name="xin")
                for kc in range(KC):
                    nc.sync.dma_start(
                        xin[:, kc * nr_all:(kc + 1) * nr_all],
                        xT_d[kc][:, r0:r0 + nr_all],
                    )
                x3 = xin[:, :KC * nr_all].rearrange("p (c r) -> p c r", c=KC)
                rhs_big = lambda kp: x3[:, 2 * kp:2 * kp + 2, 0:512]
                rhs_tail = lambda kp: x3[:, 2 * kp:2 * kp + 2, 512:576]
                return rhs_big, rhs_tail

            def rhs_of_S(S_in):
                S4 = S_in.rearrange("p (t c b) -> p c t b", t=T, c=KC)

                def f(bi):
                    t0, nst = BLOCKS[bi]
                    rhs_big = lambda kp: S4[:, 2 * kp:2 * kp + 2,
                                            t0:t0 + 32, :]
                    rhs_tail = lambda kp: S4[:, 2 * kp:2 * kp + 2,
                                             t0 + 32:t0 + 36, :]
                    return rhs_big, rhs_tail
                return f

            # ---- network
            load_w(0)
            load_w(1)
            hidden_layer(0, rhs_of_l1, S1)
            load_w(2)
            hidden_layer(1, rhs_of_S(S1), S2)
            hidden_layer(2, rhs_of_S(S2), S3)

            # ---- Output layer. Scan blocks 0-1 on gpsimd (concurrent with
            # the layer-3 DVE scan), block 2 on DVE (short tail). Spike-count
            # accumulation on gpsimd.
            wo3d = wo_sb.rearrange("p (c h) -> p c h", c=KC)
            S3_4 = S3.rearrange("p (t c b) -> p c t b", t=T, c=KC)
            for bi, (t0, nst) in enumerate(BLOCKS):
                rhs_big = lambda kp, t0=t0: S3_4[:, 2 * kp:2 * kp + 2,
                                                 t0:t0 + 32, :]
                rhs_tail = lambda kp, t0=t0: S3_4[:, 2 * kp:2 * kp + 2,
                                                  t0 + 32:t0 + 36, :]
                curo = opool.tile([128, 576], BF16, tag="curo", name="curo")

                def drain_big(hc, ps, curo=curo):
                    nc.scalar.activation(
                        curo[:100, 0:512], ps[:100, :], ACTF.Identity,
                        bias=bo_sb, scale=1.0 / WSCALE,
                    )

                def drain_tail(hc, pt, curo=curo):
                    nc.scalar.activation(
                        curo[:100, 512:576], pt[:100, :], ACTF.Identity,
                        bias=bo_sb, scale=1.0 / WSCALE,
                    )

                mm_group(wo3d, rhs_big, rhs_tail, drain_big, drain_tail,
                         1, nst)

                eng = nc.vector
                for tl in range(nst):
                    t = t0 + tl
                    cur_t = curo[:100, tl * BC:(tl + 1) * BC]
                    so_prev = so_ring[:, ((t - 1) % 8) * BC:
                                      ((t - 1) % 8 + 1) * BC]
                    so_new = so_ring[:, (t % 8) * BC:(t % 8 + 1) * BC]
                    eng.scalar_tensor_tensor(
                        memo, memo, BETA, cur_t, ALU.mult, ALU.add
                    )
                    eng.scalar_tensor_tensor(
                        so_new, memo, 1.0, so_prev, ALU.subtract, ALU.is_gt
                    )
                    eng.tensor_tensor(memo, memo, so_prev, ALU.subtract)
                    nc.gpsimd.tensor_tensor(ssum, ssum, so_new, ALU.add)

            nc.sync.dma_start(out_d[:], ssum)

    nc.compile()
    return nc


_NC_CACHE = None


def _get_nc():
    global _NC_CACHE
    if _NC_CACHE is None:
        _NC_CACHE = _build_nc()
    return _NC_CACHE


def make_in_maps(x_seq, W1, b1, W2, b2, W3, b3, Wo, bo):
    f8 = ml_dtypes.float8_e4m3
    w1 = np.ascontiguousarray((W1 * WSCALE).astype(f8))
    w2 = np.ascontiguousarray((W2 * WSCALE).astype(f8))
    w3 = np.ascontiguousarray((W3 * WSCALE).astype(f8))
    wo_pad = np.zeros((H, 128), np.float32)
    wo_pad[:, :C] = Wo * WSCALE
    wo = np.ascontiguousarray(wo_pad.astype(f8))
    biases = np.concatenate(
        [b.reshape(HC, 128).T for b in (b1, b2, b3)], axis=1
    ).astype(np.float32)                       # [128, 48]
    biases = np.ascontiguousarray(biases)
    bo_a = np.ascontiguousarray(bo.reshape(C, 1).astype(np.float32))
    in_maps = []
    for c in range(NCORES):
        xs = x_seq[:, c * BC:(c + 1) * BC, :]              # [T, BC, D]
        xT = xs.transpose(2, 0, 1).reshape(KC, 128, R)     # [D,(t,b)] chunked
        in_maps.append({
            "xT": np.ascontiguousarray(xT.astype(f8)),
            "w1": w1, "w2": w2, "w3": w3, "wo": wo,
            "biases": biases, "biaso": bo_a,
        })
    return in_maps


def kernel(x_seq, W1, b1, W2, b2, W3, b3, Wo, bo):
    nc = _get_nc()
    in_maps = make_in_maps(x_seq, W1, b1, W2, b2, W3, b3, Wo, bo)
    res = run_bass_kernel_spmd(nc, in_maps, core_ids=list(range(NCORES)))
    outs = [res.results[c]["out"] for c in range(NCORES)]   # each [C, BC]
    return np.concatenate([o.T for o in outs], axis=0).astype(np.float32)
